# revision 44
# baseline (speedup 1.0000x reference)
"""CliqueEncoder kernel for Trainium2 (8 NeuronCores, data-parallel).

Key observation: both columns of clique_attr are integers in [0, 4), so the
row-wise output depends only on (type, size) -- 16 possible rows. We fold
emb_table / W / b / gaussian basis into a 16 x 128 fp32 table on the host
(constant folding of parameters; O(1) work), and the device kernel is a pure
16-way row expansion over 1M rows:

    out[n, :] = table16[4 * attr[n, 0] + attr[n, 1], :]

Device-side per core (125,000 rows, padded to 126,976 = 2 supertiles),
scheme "tr" (feature-major output, table-stationary matmuls):
  1. gpsimd DMA attr slice in; DVE computes idx = 4*t + d as bf16 in
     [124, 512] layout (partition p holds rows [512p, 512p+512)).
  2. Per 2048-row tile t: a bf16 "replication matmul" against a 0/1
     block-select matrix EJ_t broadcasts the four 512-row idx chunks
     onto four 32-partition groups (psum[32g+j, n] = idx[4t+g, n]).
  3. One DVE is_equal against a per-partition iota of (p % 16) turns
     that into a bf16 TWO-hot [128, 512]: within each 32-row group,
     rows j and j+16 both fire when idx == j.
  4. Per group g, ONE bf16 matmul (K=32, tile_position=(32g, 0)) with
     the stationary table128 = vstack_g(bf16_hi(table16),
     bf16_lo_residual(table16)): the two-hot picks hi and lo rows in a
     single pass and the fp32 PSUM accumulate reconstructs the fp32
     table to ~2^-18 rel err (measured 7.3e-6 max vs fp32 reference).
     Output lands TRANSPOSED: psum[feature h, row n] -- so the
     stationary never changes between matmuls (no per-block one-hot
     weight loads; 5 x 512-col matmuls per tile instead of 17 matmuls)
     and PE time drops ~2.7x below the DMA roofline.
  5. ACT/DVE copy PSUM->SBUF into a [128, 8192] staging tile; every 4
     tiles one 4 MiB DMA (sync/scalar/gpsimd queues round-robin) writes
     out[h, c0:c0+8192] -- 32 KiB contiguous per partition. 1 MiB DMAs
     only sustain ~295 GB/s; 4 MiB reach the ~355 GB/s per-core HBM
     write cap, and three queues ride through per-ring completion
     stalls. The final DMA is clipped at row 125,000 (the padding
     tail is computed but never written).
  6. The host gather un-transposes dev[:, :125000].T per core.

HBM traffic per core ~ 1 MiB read + 61 MiB write -> memory-bound at the
~355 GB/s per-core cap (~178 us ideal; all-8-core contention and device
bandwidth drift put measured steady-state at ~185-215 us).
"""

import sys

sys.path.insert(0, "/opt/trn_rl_repo")

from contextlib import ExitStack

import numpy as np

# ---------------------------------------------------------------- constants
N = 1_000_000
H = 128
RBF = 32
H2 = H - H // 2  # 64
MAX_DIST = 20.0
NUM_TYPES = 4

N_CORES = 8
ROWS_PER_CORE = N // N_CORES  # 125000

F = 512  # rows per partition-chunk of a supertile
TILE_ROWS = 2048  # rows per DMA-out tile (4 groups x 512)
GROUPS = 4  # partition groups of 32 per tile


def _plan(rows_per_core):
    """Pick (p_super, tiles_per_super, n_super) covering rows_per_core."""
    rows_super_max = 128 * F  # 65536
    n_super = -(-rows_per_core // rows_super_max)
    # equal-size supertiles, padded up to a multiple of n_super * TILE_ROWS
    rows_pad = -(-rows_per_core // (n_super * TILE_ROWS)) * (n_super * TILE_ROWS)
    rows_super = rows_pad // n_super
    assert rows_super % F == 0
    p_super = rows_super // F
    tiles_per_super = rows_super // TILE_ROWS
    return p_super, tiles_per_super, n_super, rows_pad


P_SUPER, TILES_PER_SUPER, N_SUPER, ROWS_PAD = _plan(ROWS_PER_CORE)
# 124, 31, 2, 126976


# ------------------------------------------------------------- host tables
def _build_table16(emb_table, W, b):
    """table16[4*t + d] = concat(emb_table[t], basis(d) @ W[t] + b[t]).

    Computed with jax on CPU mirroring the reference ops exactly, so the
    folded table is bitwise-identical to what the reference would produce
    for each (type, size) combination.
    """
    import jax
    import jax.numpy as jnp

    cpu = jax.local_devices(backend="cpu")[0]
    with jax.default_device(cpu):
        emb_table = jnp.asarray(np.asarray(emb_table, np.float32))
        W = jnp.asarray(np.asarray(W, np.float32))
        b = jnp.asarray(np.asarray(b, np.float32))
        centers = jnp.linspace(0.0, MAX_DIST, RBF)
        std = centers[1] - centers[0]
        d = jnp.arange(NUM_TYPES, dtype=jnp.float32)
        diff = d[:, None] - centers[None, :]
        basis = jnp.exp(-0.5 * diff * diff / (std * std))  # [4, RBF]
        rows = []
        for t in range(NUM_TYPES):
            size_emb = basis @ W[t] + b[t]  # [4, H2]
            for dd in range(NUM_TYPES):
                rows.append(jnp.concatenate([emb_table[t], size_emb[dd]]))
        table = np.asarray(jnp.stack(rows), np.float32)
    return table


SCHEME = "tr"


def _build_consts(table16, tiles_per_super):
    """Device consts as an in_map fragment {table128, ejs, iota}.

    scheme "tr": table128 is bf16 with a hi/lo split packed into each
    32-row group (rows 0-15: bf16(table16), rows 16-31: bf16 residual),
    and iota is (p % 16) so the is_equal produces a TWO-hot that picks
    hi and lo rows of the stationary in a single K=32 matmul; the fp32
    PSUM accumulate reconstructs fp32 precision to ~2^-18.
    """
    ejs = np.zeros((128, tiles_per_super * 128), np.float32)
    for t in range(tiles_per_super):
        for m in range(128):
            ejs[4 * t + m // 32, t * 128 + m] = 1.0
    if SCHEME == "tr":
        import ml_dtypes

        bf = ml_dtypes.bfloat16
        t16 = np.asarray(table16, np.float32)
        hi = t16.astype(bf)
        lo = (t16 - hi.astype(np.float32)).astype(bf)
        table128 = np.zeros((128, 128), bf)
        for g in range(GROUPS):
            table128[32 * g : 32 * g + 16, :] = hi
            table128[32 * g + 16 : 32 * g + 32, :] = lo
        ejs = ejs.astype(bf)
        iota = (np.arange(128) % 16).astype(np.float32)[:, None]
    else:
        table128 = np.zeros((128, 128), np.float32)
        for g in range(GROUPS):
            table128[32 * g : 32 * g + 16, :] = table16
        iota = (np.arange(128) % 32).astype(np.float32)[:, None]
    return {"table128": table128, "ejs": ejs, "iota": iota}


# ------------------------------------------------------------ bass builder
def build_nc(
    p_super=P_SUPER,
    tiles_per_super=TILES_PER_SUPER,
    n_super=N_SUPER,
    reps=None,
    internal_io=False,
    mode="full",  # full | dma_only | no_out_dma | no_copies
    dma_tiles=4,  # 2048-row tiles batched per output DMA (1 MiB each)
    out_bufs=4,
    scheme="tr",  # tr: table-stationary matmuls, feature-major out | row
    dma3d=False,  # tr only: emit out DMA APs as [128, 16k, 128] (row-style)
    rows_valid=None,  # tr only: clip output DMA (and out dram) to this
    attr_eng="gpsimd",  # queue for the attr input DMA
    out_queues=3,  # output DMA rings: 2 = sync/scalar, 3 = +gpsimd
    dma_split=1,  # split each staged group into this many concurrent DMAs
    single_packet=False,
    taper=True,
):
    """Build the bass kernel.

    reps/internal_io are for hardware timing only: attr/out become Internal
    DRAM tensors (so no host<->device transfer dominates wall-clock) and the
    whole body is wrapped in a hardware For_i loop that runs `reps` times.
    """
    import concourse.bacc as bacc
    import concourse.bass as bass
    import concourse.mybir as mybir
    import concourse.tile as tile

    f32 = mybir.dt.float32
    f32r = mybir.dt.float32r
    bf16 = mybir.dt.bfloat16
    i32 = mybir.dt.int32
    rows_super = p_super * F
    rows_pad = n_super * rows_super

    nc = bacc.Bacc(None, target_bir_lowering=False)

    io_kind = "Internal" if internal_io else None
    attr_d = nc.dram_tensor(
        "attr", [rows_pad, 2], i32, kind=io_kind or "ExternalInput"
    )
    tbl_dt = bf16 if scheme == "tr" else f32
    tbl_d = nc.dram_tensor("table128", [128, 128], tbl_dt, kind="ExternalInput")
    ejs_dt = bf16 if scheme == "tr" else f32
    ejs_d = nc.dram_tensor(
        "ejs", [128, tiles_per_super * 128], ejs_dt, kind="ExternalInput"
    )
    iota_d = nc.dram_tensor("iota", [128, 1], f32, kind="ExternalInput")
    # Output layout:
    # - scheme "row": partition-major [128, rows_pad // 128, H]: out_dev
    #   [m, b, :] holds logical row 128*b + m -> 8 KiB contiguous DRAM
    #   chunks per partition per 1 MiB tile.
    # - scheme "tr": feature-major [128, rows_pad]: partition h holds
    #   feature h of every row. Selection matmuls keep the (constant)
    #   table as the stationary operand streaming 512-row one-hot blocks
    #   (f32r -> 1 cyc/col), and each output DMA is one fully contiguous
    #   chunk per partition. The host transposes during the gather copy.
    n_blocks = rows_pad // 128
    if rows_valid is None or scheme != "tr":
        rows_valid = rows_pad
    if scheme == "tr":
        # keep the dram tensor padded: per-partition stride stays 4 KiB-
        # aligned (507904 B); rows_valid only clips the final DMA width
        out_d = nc.dram_tensor(
            "out", [128, rows_pad], f32, kind=io_kind or "ExternalOutput"
        )
    else:
        out_d = nc.dram_tensor(
            "out", [128, n_blocks, H], f32, kind=io_kind or "ExternalOutput"
        )
    dummy_d = (
        nc.dram_tensor("probe", [128, 128], tbl_dt, kind="ExternalOutput")
        if internal_io
        else None
    )

    with tile.TileContext(nc) as tc, ExitStack() as ctx:
        const_p = ctx.enter_context(tc.tile_pool(name="const", bufs=1))
        attr_p = ctx.enter_context(tc.tile_pool(name="attr", bufs=2))
        idx_p = ctx.enter_context(tc.tile_pool(name="idx", bufs=2))
        scr_p = ctx.enter_context(tc.tile_pool(name="scr", bufs=2))
        oh_p = ctx.enter_context(tc.tile_pool(name="oh", bufs=4))
        out_p = ctx.enter_context(tc.tile_pool(name="out", bufs=out_bufs))
        psi_p = ctx.enter_context(
            tc.tile_pool(name="psi", bufs=2, space=bass.MemorySpace.PSUM)
        )
        pso_p = ctx.enter_context(
            tc.tile_pool(name="pso", bufs=3, space=bass.MemorySpace.PSUM)
        )

        # consts on separate queues so they load concurrently with the
        # first attr slice (gpsimd) during the startup ramp
        tbl = const_p.tile([128, 128], tbl_dt)
        nc.sync.dma_start(tbl[:], tbl_d[:, :])
        iota = const_p.tile([128, 1], f32)
        nc.sync.dma_start(iota[:], iota_d[:, :])
        # split the ejs load so tile 0 (which reads only cols [0,128))
        # doesn't wait for the whole 1 MiB constant during the ramp
        ejs = const_p.tile([128, tiles_per_super * 128], ejs_dt)
        nc.scalar.dma_start(ejs[:, :128], ejs_d[:, :128])
        nc.scalar.dma_start(ejs[:, 128:], ejs_d[:, 128:])

        idx_dt = bf16 if scheme == "tr" else f32

        def emit_supertile(s):
            attr3 = attr_p.tile([p_super, F, 2], i32, name=f"attr3_{s}")
            getattr(nc, attr_eng).dma_start(
                attr3[:],
                attr_d[s * rows_super : (s + 1) * rows_super, :].rearrange(
                    "(p f) c -> p f c", p=p_super
                ),
            )
            idx_t = idx_p.tile([128, F], idx_dt)
            if p_super < 128:
                nc.vector.memset(idx_t[:], 0.0)
            t4 = scr_p.tile([p_super, F], idx_dt)
            nc.vector.tensor_scalar(
                t4[:], attr3[:, :, 0], 4, None, mybir.AluOpType.mult
            )
            dv = scr_p.tile([p_super, F], idx_dt)
            nc.vector.tensor_copy(dv[:], attr3[:, :, 1])
            nc.vector.tensor_add(idx_t[:p_super, :], t4[:], dv[:])

            # taper the group schedule: small first groups let the first
            # output DMA launch after ~1 tile of compute (shorter ramp),
            # small last groups shrink the post-compute drain tail
            if mode == "full" and dma_tiles >= 3 and taper:
                sched = [1, 2]
                body = tiles_per_super - 7
                sched += [dma_tiles] * (body // dma_tiles)
                rem = body % dma_tiles
                if rem:
                    sched.append(rem)
                sched += [2, 2]
            else:
                sched = [dma_tiles] * (-(-tiles_per_super // dma_tiles))
                sched[-1] -= sum(sched) - tiles_per_super
            starts = [sum(sched[:i]) for i in range(len(sched))]
            for gi, (t0, k) in enumerate(zip(starts, sched)):
                if scheme == "tr":
                    out_sb = out_p.tile([128, dma_tiles * TILE_ROWS], f32)
                else:
                    out_sb = out_p.tile([128, dma_tiles * 16, 128], f32)
                if mode == "dma_only":
                    # touch the tile so Tile materializes it
                    corner = (
                        out_sb[:, 0:4] if scheme == "tr" else out_sb[:, 0:1, 0:4]
                    )
                    nc.vector.memset(corner, 0.0)
                for t in range(t0, t0 + k) if mode != "dma_only" else []:
                    tt = t - t0
                    ps_idx = psi_p.tile([128, F], f32)
                    nc.tensor.matmul(
                        ps_idx[:],
                        ejs[:, t * 128 : (t + 1) * 128],
                        idx_t[:],
                        start=True,
                        stop=True,
                    )
                    oh = oh_p.tile([128, F], bf16 if scheme == "tr" else f32)
                    nc.vector.tensor_scalar(
                        oh[:], ps_idx[:], iota[:], None, mybir.AluOpType.is_equal
                    )

                    # two 2-bank PSUM tiles per 2048-row tile: halves the
                    # PSUM->SBUF copy count (per-op overhead is ~230 ns)
                    if scheme == "tr":
                        ps_outs = [
                            pso_p.tile([128, 2 * F], f32, tag="pso", name=f"pso{G}")
                            for G in range(2)
                        ]
                        for g in range(GROUPS):
                            nc.tensor.matmul(
                                ps_outs[g // 2][:, (g % 2) * F : (g % 2 + 1) * F],
                                tbl[32 * g : 32 * g + 32, :],
                                oh[32 * g : 32 * g + 32, :],
                                start=True,
                                stop=True,
                                tile_position=(32 * g, 0),
                            )
                    else:
                        ps_outs = [
                            pso_p.tile([128, 8, 128], f32, tag="pso", name=f"pso{G}")
                            for G in range(2)
                        ]
                        for j in range(4):
                            for g in range(GROUPS):
                                nc.tensor.matmul(
                                    ps_outs[g // 2][:, 4 * (g % 2) + j, :],
                                    oh[32 * g : 32 * g + 32, j * 128 : (j + 1) * 128],
                                    tbl[32 * g : 32 * g + 32, :],
                                    start=True,
                                    stop=True,
                                    tile_position=(32 * g, 0),
                                )
                    if mode != "no_copies":
                        # DVE also does the one-hot op; give ACT slightly
                        # more of the copy work (x2 on every 3rd tile).
                        for G in range(2):
                            if scheme == "tr":
                                dst = out_sb[
                                    :,
                                    TILE_ROWS * tt + 2 * F * G : TILE_ROWS * tt
                                    + 2 * F * (G + 1),
                                ]
                            else:
                                dst = out_sb[
                                    :, 16 * tt + 8 * G : 16 * tt + 8 * G + 8, :
                                ]
                            if G == 0 and t % 3 != 2:
                                nc.vector.tensor_copy(dst, ps_outs[G][:])
                            else:
                                nc.scalar.copy(dst, ps_outs[G][:])

                if mode != "no_out_dma":
                    engs = [nc.sync, nc.scalar, nc.gpsimd][:out_queues]
                    if scheme == "tr":
                        c0 = s * rows_super + t0 * TILE_ROWS
                        w = min(k * TILE_ROWS, rows_valid - c0)
                        # split the staged group into dma_split chunks on
                        # distinct queues so they drain concurrently
                        wc = -(-w // dma_split)
                        for j in range(dma_split):
                            lo = j * wc
                            hi = min(w, lo + wc)
                            if hi <= lo:
                                break
                            eng = engs[(gi * dma_split + j) % len(engs)]
                            dst = out_d[:, c0 + lo : c0 + hi]
                            src = out_sb[:, lo:hi]
                            if dma3d:
                                dst = dst.rearrange("p (B j) -> p B j", j=128)
                                src = src.rearrange("p (B j) -> p B j", j=128)
                            eng.dma_start(dst, src, single_packet=single_packet)
                    else:
                        b0 = (s * rows_super + t0 * TILE_ROWS) // 128
                        engs[gi % len(engs)].dma_start(
                            out_d[:, b0 : b0 + 16 * k, :], out_sb[:, : 16 * k, :]
                        )

        def emit_body():
            for s in range(n_super):
                emit_supertile(s)

        if reps is None:
            emit_body()
        else:
            with tc.For_i(0, reps, 1, hint_engines=tuple(mybir.ALL_ENGINES)):
                emit_body()

        if dummy_d is not None:
            nc.sync.dma_start(dummy_d[:, :], tbl[:])

    nc.compile()
    return nc


# --------------------------------------------------------------- host entry
_CACHE = {}


def _get_nc():
    if "nc" not in _CACHE:
        _CACHE["nc"] = build_nc(scheme=SCHEME, rows_valid=ROWS_PER_CORE)
    return _CACHE["nc"]


def kernel(clique_attr, emb_table, W, b):
    from concourse.bass_utils import run_bass_kernel_spmd

    clique_attr = np.ascontiguousarray(np.asarray(clique_attr, np.int32))
    table16 = _build_table16(emb_table, W, b)
    consts = _build_consts(table16, TILES_PER_SUPER)

    nc = _get_nc()
    in_maps = []
    for c in range(N_CORES):
        sl = clique_attr[c * ROWS_PER_CORE : (c + 1) * ROWS_PER_CORE]
        pad = np.zeros((ROWS_PAD, 2), np.int32)
        pad[: len(sl)] = sl
        in_maps.append({"attr": pad, **consts})

    res = run_bass_kernel_spmd(nc, in_maps, core_ids=list(range(N_CORES)))
    out = np.empty((N, H), np.float32)
    for c in range(N_CORES):
        dev = res.results[c]["out"]
        if SCHEME == "tr":
            # device layout [128, rows]: feature h of row r at [h, r]
            out[c * ROWS_PER_CORE : (c + 1) * ROWS_PER_CORE] = dev[
                :, :ROWS_PER_CORE
            ].T
        else:
            # device layout [128, n_blocks, H]: row 128*b+m lives at [m, b, :]
            rows = dev.transpose(1, 0, 2).reshape(-1, H)
            out[c * ROWS_PER_CORE : (c + 1) * ROWS_PER_CORE] = rows[
                :ROWS_PER_CORE
            ]
    return out

